# Initial kernel scaffold
#
"""CRPS loss kernel for Trainium2, 8 NeuronCores (SPMD data-parallel).

reference semantics:
    p, t = prediction.ravel(), target.ravel()       # N = 16,611,840 each
    lo, hi = min(min p, min t), max(max p, max t)
    x = linspace(lo, hi, 1000)  (f32)
    cdf_q(x_i) = #{v in q : v <= x_i} / N
    return trapz(|cdf_p - cdf_t|^2, x)

Device work (per core, 1/8 shard of each tensor):
  kernel A: running min/max reduce  -> per-core (min, -max)
  kernel B: per element j = ceil((v-lo)/dx) via round-to-nearest-even cast
            (j = rint(v*A + B), A = 1/dx, B = -lo*A + 0.5), split j = 32*a+b,
            build bin-major bf16 one-hots with 64 tensor_scalar(is_equal) ops,
            accumulate joint histogram M[32,32] = sum_e onehot32(a) x onehot32(b)
            via one PE matmul per 128-element group into PSUM.
Host: combine 8 cores' histograms, fold j>=999 into bin 999, cumsum -> exact
      searchsorted counts at every x_i, then the 1000-point trapz in f64.

Shards are padded with the shard's first element to [128, 16384]; the host
subtracts the pad count from the padded value's bin (exact, same f32 math).
"""

import numpy as np
from concourse import bacc, mybir, tile
from concourse.bass_utils import run_bass_kernel_spmd

P = 128
NCORES = 8
TOTAL = 16 * 1 * 721 * 1440          # 16,611,840
SHARD = TOTAL // NCORES              # 2,076,480
KTOT = 16384                         # padded columns/core/tensor (P*KTOT = 2,097,152)
PADN = P * KTOT - SHARD              # 20,672
NB = 32                              # 32x32 = 1024 bins
NX = 1000
CHUNK = 512
NCHUNK = KTOT // CHUNK               # 32
PACK = 1                             # element-groups packed per matmul
RED_CHUNK = 2048
F32 = mybir.dt.float32
I32 = mybir.dt.int32
BF16 = mybir.dt.bfloat16
ALU = mybir.AluOpType


def _build_minmax():
    nc = bacc.Bacc()
    ins = [
        nc.declare_dram_parameter("pv", [P, KTOT], F32, isOutput=False),
        nc.declare_dram_parameter("tv", [P, KTOT], F32, isOutput=False),
    ]
    out = nc.declare_dram_parameter("mm", [1, 2], F32, isOutput=True)  # (-min, max)

    with tile.TileContext(nc) as tc:
        with (
            tc.tile_pool(name="sbuf", bufs=4) as pool,
            tc.tile_pool(name="acc", bufs=1) as apool,
        ):
            nred = (KTOT // RED_CHUNK) * 2
            mins = apool.tile([P, nred], F32)
            maxs = apool.tile([P, nred], F32)
            col = 0
            for src in ins:
                for ci in range(KTOT // RED_CHUNK):
                    v = pool.tile([P, RED_CHUNK], F32, tag="v")
                    nc.sync.dma_start(v[:], src[:, ci * RED_CHUNK:(ci + 1) * RED_CHUNK])
                    nc.vector.tensor_reduce(
                        mins[:, col:col + 1], v[:], mybir.AxisListType.X, ALU.min)
                    nc.vector.tensor_reduce(
                        maxs[:, col:col + 1], v[:], mybir.AxisListType.X, ALU.max)
                    col += 1
            pmin = apool.tile([P, 1], F32)
            pmax = apool.tile([P, 1], F32)
            nc.vector.tensor_reduce(pmin[:], mins[:], mybir.AxisListType.X, ALU.min)
            nc.vector.tensor_reduce(pmax[:], maxs[:], mybir.AxisListType.X, ALU.max)
            # cross-lane reduce only supports add/average/max -> store (-min, max)
            both = apool.tile([P, 2], F32)
            nc.vector.tensor_scalar(out=both[:, 0:1], in0=pmin[:], scalar1=-1.0,
                                    scalar2=None, op0=ALU.mult)
            nc.vector.tensor_copy(out=both[:, 1:2], in_=pmax[:])
            red = apool.tile([1, 2], F32)
            nc.gpsimd.tensor_reduce(red[:], both[:], mybir.AxisListType.C, ALU.max)
            nc.sync.dma_start(out[:], red[:])
    nc.compile()
    return nc


def _build_hist():
    nc = bacc.Bacc()
    ins = [
        nc.declare_dram_parameter("pv", [P, KTOT], F32, isOutput=False),
        nc.declare_dram_parameter("tv", [P, KTOT], F32, isOutput=False),
    ]
    ab_in = nc.declare_dram_parameter("ab", [P, 2], F32, isOutput=False)
    # hist[a, t*NB + b]: t in {0: prediction, 1: target}
    out = nc.declare_dram_parameter("hist", [NB, 2 * NB], F32, isOutput=True)

    with tile.TileContext(nc) as tc:
        with (
            tc.tile_pool(name="sbuf", bufs=3) as pool,
            tc.tile_pool(name="oh", bufs=2) as ohpool,
            tc.tile_pool(name="const", bufs=1) as cpool,
            tc.tile_pool(name="acc", bufs=1) as apool,
            tc.tile_pool(name="psum", bufs=4, space="PSUM") as psum_pool,
        ):
            ab_raw = cpool.tile([P, 2], F32)
            nc.sync.dma_start(ab_raw[:], ab_in[:])
            # DVE-bounce so tensor_scalar consumers dep on a same-engine producer
            ab = cpool.tile([P, 2], F32)
            nc.vector.tensor_copy(out=ab[:], in_=ab_raw[:])

            hacc = apool.tile([NB, 2 * NB], F32)
            nc.vector.memset(hacc[:], 0.0)

            # drains deferred one chunk so the DVE's psum->hacc add never
            # blocks the next chunk's one-hot build on the critical path
            pending = []  # (ti, m_psum)

            def drain_pending():
                while pending:
                    pti, pm = pending.pop(0)
                    # pm is [2*NB, 2*NB]; diagonal NBxNB blocks are the two
                    # packed groups' histograms
                    for blk in range(PACK):
                        nc.vector.tensor_tensor(
                            out=hacc[:, pti * NB:(pti + 1) * NB],
                            in0=hacc[:, pti * NB:(pti + 1) * NB],
                            in1=pm[blk * NB:(blk + 1) * NB, blk * NB:(blk + 1) * NB],
                            op=ALU.add,
                        )

            for ti, src in enumerate(ins):
                for ci in range(NCHUNK):
                    v = pool.tile([P, CHUNK], F32, tag="v")
                    nc.sync.dma_start(v[:], src[:, ci * CHUNK:(ci + 1) * CHUNK])
                    t1 = pool.tile([P, CHUNK], F32, tag="t1")
                    nc.vector.tensor_scalar(out=t1[:], in0=v[:], scalar1=ab[:, 0:1],
                                            scalar2=None, op0=ALU.mult)
                    zf = pool.tile([P, CHUNK], F32, tag="zf")
                    nc.vector.tensor_scalar(out=zf[:], in0=t1[:], scalar1=ab[:, 1:2],
                                            scalar2=None, op0=ALU.add)
                    ji = pool.tile([P, CHUNK], I32, tag="ji")
                    nc.vector.tensor_copy(out=ji[:], in_=zf[:])  # rint (round-even)
                    ai32 = pool.tile([P, CHUNK], I32, tag="ai32")
                    nc.vector.tensor_scalar(out=ai32[:], in0=ji[:], scalar1=5,
                                            scalar2=None, op0=ALU.arith_shift_right)
                    bi32 = pool.tile([P, CHUNK], I32, tag="bi32")
                    nc.vector.tensor_scalar(out=bi32[:], in0=ji[:], scalar1=31,
                                            scalar2=None, op0=ALU.bitwise_and)
                    # bf16 digits (0..31 exact): 16-bit in/out is_equal packs
                    ai = pool.tile([P, CHUNK], BF16, tag="ai")
                    nc.vector.tensor_copy(out=ai[:], in_=ai32[:])
                    bi = pool.tile([P, CHUNK], BF16, tag="bi")
                    nc.vector.tensor_copy(out=bi[:], in_=bi32[:])
                    # bin-major one-hots: oh[p, q*CHUNK + e] = (idx[p,e] == q)
                    oh_a = ohpool.tile([P, NB * CHUNK], BF16, tag="oh_a")
                    oh_b = ohpool.tile([P, NB * CHUNK], BF16, tag="oh_b")
                    for q in range(NB):
                        nc.vector.tensor_scalar(
                            out=oh_a[:, q * CHUNK:(q + 1) * CHUNK], in0=ai[:],
                            scalar1=float(q), scalar2=None, op0=ALU.is_equal)
                        nc.vector.tensor_scalar(
                            out=oh_b[:, q * CHUNK:(q + 1) * CHUNK], in0=bi[:],
                            scalar1=float(q), scalar2=None, op0=ALU.is_equal)
                    # PACK adjacent element-groups per matmul: lhsT/rhs
                    # [128, PACK*NB] via 3D AP [[1,PACK],[CHUNK,NB]]; out
                    # [PACK*NB, PACK*NB], diag blocks = per-group joint hists
                    oh_a4 = oh_a[:].rearrange("p (q e g) -> p e g q", q=NB, g=PACK)
                    oh_b4 = oh_b[:].rearrange("p (q e g) -> p e g q", q=NB, g=PACK)
                    m_psum = psum_pool.tile([PACK * NB, PACK * NB], F32,
                                            space="PSUM", tag="m")
                    ngrp = CHUNK // PACK
                    for e in range(ngrp):
                        nc.tensor.matmul(
                            m_psum[:],
                            lhsT=oh_a4[:, e],
                            rhs=oh_b4[:, e],
                            start=(e == 0),
                            stop=(e == ngrp - 1),
                        )
                    drain_pending()
                    pending.append((ti, m_psum))
            drain_pending()
            nc.sync.dma_start(out[:], hacc[:])
    nc.compile()
    return nc


_KERNELS = {}


def _get_kernels():
    if "mm" not in _KERNELS:
        _KERNELS["mm"] = _build_minmax()
        _KERNELS["hist"] = _build_hist()
    return _KERNELS["mm"], _KERNELS["hist"]


def _shard(flat):
    """Split [TOTAL] -> per-core padded [P, KTOT] tiles + pad values."""
    tiles, pads = [], []
    for c in range(NCORES):
        s = flat[c * SHARD:(c + 1) * SHARD]
        v0 = s[0]
        t = np.concatenate([s, np.full(PADN, v0, s.dtype)]).reshape(P, KTOT)
        tiles.append(t)
        pads.append(v0)
    return tiles, pads


def _bin_of(v, A, B):
    """Replicate device binning for a scalar f32 value."""
    t1 = np.float32(np.float32(v) * A)
    z = np.float32(t1 + B)
    j = int(np.rint(np.float64(z)))
    return min(max(j, 0), NB * NB - 1)


def kernel(prediction, target):
    nc_mm, nc_hist = _get_kernels()
    p = np.ascontiguousarray(np.asarray(prediction, dtype=np.float32).ravel())
    t = np.ascontiguousarray(np.asarray(target, dtype=np.float32).ravel())
    p_tiles, p_pads = _shard(p)
    t_tiles, t_pads = _shard(t)
    core_ids = list(range(NCORES))

    in_maps = [{"pv": p_tiles[c], "tv": t_tiles[c]} for c in core_ids]
    res = run_bass_kernel_spmd(nc_mm, in_maps, core_ids).results
    mm = np.stack([r["mm"][0] for r in res])        # [8, 2] = (-min, max)
    lo = np.float32(-(mm[:, 0].max()))
    hi = np.float32(mm[:, 1].max())

    dx = np.float32((hi - lo) / np.float32(NX - 1))
    A = np.float32(np.float32(1.0) / dx)
    B = np.float32(np.float32(-lo * A) + np.float32(0.5))
    ab = np.stack([np.full(P, A, np.float32), np.full(P, B, np.float32)], axis=1)

    in_maps = [{"pv": p_tiles[c], "tv": t_tiles[c], "ab": ab} for c in core_ids]
    res = run_bass_kernel_spmd(nc_hist, in_maps, core_ids).results

    hp = np.zeros(NB * NB, np.float64)
    ht = np.zeros(NB * NB, np.float64)
    for c in core_ids:
        h = res[c]["hist"].astype(np.float64)      # [NB, 2*NB]
        hp += h[:, :NB].ravel()
        ht += h[:, NB:].ravel()
        hp[_bin_of(p_pads[c], A, B)] -= PADN
        ht[_bin_of(t_pads[c], A, B)] -= PADN

    # fold j >= NX-1 into bin NX-1, cumsum -> counts at x_i
    hp[NX - 1] += hp[NX:].sum()
    ht[NX - 1] += ht[NX:].sum()
    cnt_p = np.cumsum(hp[:NX])
    cnt_t = np.cumsum(ht[:NX])

    n = np.float64(TOTAL)
    diff = np.abs(cnt_p / n - cnt_t / n)
    y = diff * diff
    x = np.linspace(np.float64(lo), np.float64(hi), NX)
    dxs = x[1:] - x[:-1]
    out = np.sum(0.5 * (y[1:] + y[:-1]) * dxs)
    return np.float32(out)



# revision 1
# speedup vs baseline: 2.8093x; 2.8093x over previous
"""CRPS loss kernel for Trainium2, 8 NeuronCores (SPMD data-parallel).

reference semantics:
    p, t = prediction.ravel(), target.ravel()       # N = 16,611,840 each
    lo, hi = min(min p, min t), max(max p, max t)
    x = linspace(lo, hi, 1000)  (f32)
    cdf_q(x_i) = #{v in q : v <= x_i} / N
    return trapz(|cdf_p - cdf_t|^2, x)

Device work (per core, 1/8 shard of each tensor):
  kernel A: running min/max reduce  -> per-core (min, -max)
  kernel B: per element j = ceil((v-lo)/dx) via round-to-nearest-even cast
            (j = rint(v*A + B), A = 1/dx, B = -lo*A + 0.5), split j = 32*a+b,
            build bin-major bf16 one-hots with 64 tensor_scalar(is_equal) ops,
            accumulate joint histogram M[32,32] = sum_e onehot32(a) x onehot32(b)
            via one PE matmul per 128-element group into PSUM.
Host: combine 8 cores' histograms, fold j>=999 into bin 999, cumsum -> exact
      searchsorted counts at every x_i, then the 1000-point trapz in f64.

Shards are padded with the shard's first element to [128, 16384]; the host
subtracts the pad count from the padded value's bin (exact, same f32 math).
"""

import numpy as np
from concourse import bacc, mybir, tile
from concourse.bass_utils import run_bass_kernel_spmd

P = 128
NCORES = 8
TOTAL = 16 * 1 * 721 * 1440          # 16,611,840
SHARD = TOTAL // NCORES              # 2,076,480
KTOT = 16384                         # padded columns/core/tensor (P*KTOT = 2,097,152)
PADN = P * KTOT - SHARD              # 20,672
NB = 32                              # 32x32 = 1024 bins
NX = 1000
CHUNK = 512
NCHUNK = KTOT // CHUNK               # 32
PACK = 1                             # element-groups packed per matmul
RED_CHUNK = 2048
F32 = mybir.dt.float32
I32 = mybir.dt.int32
BF16 = mybir.dt.bfloat16
ALU = mybir.AluOpType


def _build_minmax():
    nc = bacc.Bacc()
    ins = [
        nc.declare_dram_parameter("pv", [P, KTOT], F32, isOutput=False),
        nc.declare_dram_parameter("tv", [P, KTOT], F32, isOutput=False),
    ]
    out = nc.declare_dram_parameter("mm", [1, 2], F32, isOutput=True)  # (-min, max)

    with tile.TileContext(nc) as tc:
        with (
            tc.tile_pool(name="sbuf", bufs=4) as pool,
            tc.tile_pool(name="acc", bufs=1) as apool,
        ):
            nred = (KTOT // RED_CHUNK) * 2
            mins = apool.tile([P, nred], F32)
            maxs = apool.tile([P, nred], F32)
            col = 0
            for src in ins:
                for ci in range(KTOT // RED_CHUNK):
                    v = pool.tile([P, RED_CHUNK], F32, tag="v")
                    nc.sync.dma_start(v[:], src[:, ci * RED_CHUNK:(ci + 1) * RED_CHUNK])
                    nc.vector.tensor_reduce(
                        mins[:, col:col + 1], v[:], mybir.AxisListType.X, ALU.min)
                    nc.vector.tensor_reduce(
                        maxs[:, col:col + 1], v[:], mybir.AxisListType.X, ALU.max)
                    col += 1
            pmin = apool.tile([P, 1], F32)
            pmax = apool.tile([P, 1], F32)
            nc.vector.tensor_reduce(pmin[:], mins[:], mybir.AxisListType.X, ALU.min)
            nc.vector.tensor_reduce(pmax[:], maxs[:], mybir.AxisListType.X, ALU.max)
            # cross-lane reduce only supports add/average/max -> store (-min, max)
            both = apool.tile([P, 2], F32)
            nc.vector.tensor_scalar(out=both[:, 0:1], in0=pmin[:], scalar1=-1.0,
                                    scalar2=None, op0=ALU.mult)
            nc.vector.tensor_copy(out=both[:, 1:2], in_=pmax[:])
            red = apool.tile([1, 2], F32)
            nc.gpsimd.tensor_reduce(red[:], both[:], mybir.AxisListType.C, ALU.max)
            nc.sync.dma_start(out[:], red[:])
    nc.compile()
    return nc


def _build_hist():
    nc = bacc.Bacc()
    ins = [
        nc.declare_dram_parameter("pv", [P, KTOT], F32, isOutput=False),
        nc.declare_dram_parameter("tv", [P, KTOT], F32, isOutput=False),
    ]
    ab_in = nc.declare_dram_parameter("ab", [P, 2], F32, isOutput=False)
    # hist[a, t*NB + b]: t in {0: prediction, 1: target}
    out = nc.declare_dram_parameter("hist", [NB, 2 * NB], F32, isOutput=True)

    with tile.TileContext(nc) as tc:
        with (
            tc.tile_pool(name="sbuf", bufs=3) as pool,
            tc.tile_pool(name="oh", bufs=2) as ohpool,
            tc.tile_pool(name="const", bufs=1) as cpool,
            tc.tile_pool(name="acc", bufs=1) as apool,
            tc.tile_pool(name="psum", bufs=4, space="PSUM") as psum_pool,
        ):
            ab_raw = cpool.tile([P, 2], F32)
            nc.sync.dma_start(ab_raw[:], ab_in[:])
            # DVE-bounce so tensor_scalar consumers dep on a same-engine producer
            ab = cpool.tile([P, 2], F32)
            nc.vector.tensor_copy(out=ab[:], in_=ab_raw[:])

            hacc = apool.tile([NB, 2 * NB], F32)
            nc.vector.memset(hacc[:], 0.0)

            # drains deferred one chunk so the DVE's psum->hacc add never
            # blocks the next chunk's one-hot build on the critical path
            pending = []  # (ti, m_psum)

            def drain_pending():
                while pending:
                    pti, pm = pending.pop(0)
                    # pm is [2*NB, 2*NB]; diagonal NBxNB blocks are the two
                    # packed groups' histograms
                    for blk in range(PACK):
                        nc.vector.tensor_tensor(
                            out=hacc[:, pti * NB:(pti + 1) * NB],
                            in0=hacc[:, pti * NB:(pti + 1) * NB],
                            in1=pm[blk * NB:(blk + 1) * NB, blk * NB:(blk + 1) * NB],
                            op=ALU.add,
                        )

            for ti, src in enumerate(ins):
                for ci in range(NCHUNK):
                    v = pool.tile([P, CHUNK], F32, tag="v")
                    nc.sync.dma_start(v[:], src[:, ci * CHUNK:(ci + 1) * CHUNK])
                    t1 = pool.tile([P, CHUNK], F32, tag="t1")
                    nc.vector.tensor_scalar(out=t1[:], in0=v[:], scalar1=ab[:, 0:1],
                                            scalar2=None, op0=ALU.mult)
                    zf = pool.tile([P, CHUNK], F32, tag="zf")
                    nc.vector.tensor_scalar(out=zf[:], in0=t1[:], scalar1=ab[:, 1:2],
                                            scalar2=None, op0=ALU.add)
                    ji = pool.tile([P, CHUNK], I32, tag="ji")
                    nc.vector.tensor_copy(out=ji[:], in_=zf[:])  # rint (round-even)
                    ai32 = pool.tile([P, CHUNK], I32, tag="ai32")
                    nc.vector.tensor_scalar(out=ai32[:], in0=ji[:], scalar1=5,
                                            scalar2=None, op0=ALU.arith_shift_right)
                    bi32 = pool.tile([P, CHUNK], I32, tag="bi32")
                    nc.vector.tensor_scalar(out=bi32[:], in0=ji[:], scalar1=31,
                                            scalar2=None, op0=ALU.bitwise_and)
                    # bf16 digits (0..31 exact): 16-bit in/out is_equal packs
                    ai = pool.tile([P, CHUNK], BF16, tag="ai")
                    nc.vector.tensor_copy(out=ai[:], in_=ai32[:])
                    bi = pool.tile([P, CHUNK], BF16, tag="bi")
                    nc.vector.tensor_copy(out=bi[:], in_=bi32[:])
                    # bin-major one-hots: oh[p, q*CHUNK + e] = (idx[p,e] == q)
                    oh_a = ohpool.tile([P, NB * CHUNK], BF16, tag="oh_a")
                    oh_b = ohpool.tile([P, NB * CHUNK], BF16, tag="oh_b")
                    for q in range(NB):
                        nc.vector.tensor_scalar(
                            out=oh_a[:, q * CHUNK:(q + 1) * CHUNK], in0=ai[:],
                            scalar1=float(q), scalar2=None, op0=ALU.is_equal)
                        nc.vector.tensor_scalar(
                            out=oh_b[:, q * CHUNK:(q + 1) * CHUNK], in0=bi[:],
                            scalar1=float(q), scalar2=None, op0=ALU.is_equal)
                    # PACK adjacent element-groups per matmul: lhsT/rhs
                    # [128, PACK*NB] via 3D AP [[1,PACK],[CHUNK,NB]]; out
                    # [PACK*NB, PACK*NB], diag blocks = per-group joint hists
                    oh_a4 = oh_a[:].rearrange("p (q e g) -> p e g q", q=NB, g=PACK)
                    oh_b4 = oh_b[:].rearrange("p (q e g) -> p e g q", q=NB, g=PACK)
                    m_psum = psum_pool.tile([PACK * NB, PACK * NB], F32,
                                            space="PSUM", tag="m")
                    ngrp = CHUNK // PACK
                    for e in range(ngrp):
                        nc.tensor.matmul(
                            m_psum[:],
                            lhsT=oh_a4[:, e],
                            rhs=oh_b4[:, e],
                            start=(e == 0),
                            stop=(e == ngrp - 1),
                        )
                    drain_pending()
                    pending.append((ti, m_psum))
            drain_pending()
            nc.sync.dma_start(out[:], hacc[:])
    nc.compile()
    return nc


_KERNELS = {}


def _get_kernels():
    if "mm" not in _KERNELS:
        _KERNELS["mm"] = _build_minmax()
        _KERNELS["hist"] = _build_hist()
    return _KERNELS["mm"], _KERNELS["hist"]


def _shard(flat):
    """Split [TOTAL] -> per-core padded [P, KTOT] tiles + pad values."""
    tiles, pads = [], []
    for c in range(NCORES):
        s = flat[c * SHARD:(c + 1) * SHARD]
        v0 = s[0]
        t = np.concatenate([s, np.full(PADN, v0, s.dtype)]).reshape(P, KTOT)
        tiles.append(t)
        pads.append(v0)
    return tiles, pads


def _bin_of(v, A, B):
    """Replicate device binning for a scalar f32 value."""
    t1 = np.float32(np.float32(v) * A)
    z = np.float32(t1 + B)
    j = int(np.rint(np.float64(z)))
    return min(max(j, 0), NB * NB - 1)


def kernel(prediction, target):
    nc_mm, nc_hist = _get_kernels()
    p = np.ascontiguousarray(np.asarray(prediction, dtype=np.float32).ravel())
    t = np.ascontiguousarray(np.asarray(target, dtype=np.float32).ravel())
    p_tiles, p_pads = _shard(p)
    t_tiles, t_pads = _shard(t)
    core_ids = list(range(NCORES))

    in_maps = [{"pv": p_tiles[c], "tv": t_tiles[c]} for c in core_ids]
    res = run_bass_kernel_spmd(nc_mm, in_maps, core_ids).results
    mm = np.stack([r["mm"][0] for r in res])        # [8, 2] = (-min, max)
    lo = np.float32(-(mm[:, 0].max()))
    hi = np.float32(mm[:, 1].max())

    dx = np.float32((hi - lo) / np.float32(NX - 1))
    A = np.float32(np.float32(1.0) / dx)
    B = np.float32(np.float32(-lo * A) + np.float32(0.5))
    ab = np.stack([np.full(P, A, np.float32), np.full(P, B, np.float32)], axis=1)

    in_maps = [{"pv": p_tiles[c], "tv": t_tiles[c], "ab": ab} for c in core_ids]
    res = run_bass_kernel_spmd(nc_hist, in_maps, core_ids).results

    hp = np.zeros(NB * NB, np.float64)
    ht = np.zeros(NB * NB, np.float64)
    for c in core_ids:
        h = res[c]["hist"].astype(np.float64)      # [NB, 2*NB]
        hp += h[:, :NB].ravel()
        ht += h[:, NB:].ravel()
        hp[_bin_of(p_pads[c], A, B)] -= PADN
        ht[_bin_of(t_pads[c], A, B)] -= PADN

    # fold j >= NX-1 into bin NX-1, cumsum -> counts at x_i
    hp[NX - 1] += hp[NX:].sum()
    ht[NX - 1] += ht[NX:].sum()
    cnt_p = np.cumsum(hp[:NX])
    cnt_t = np.cumsum(ht[:NX])

    n = np.float64(TOTAL)
    diff = np.abs(cnt_p / n - cnt_t / n)
    y = diff * diff
    x = np.linspace(np.float64(lo), np.float64(hi), NX)
    dxs = x[1:] - x[:-1]
    out = np.sum(0.5 * (y[1:] + y[:-1]) * dxs)
    return np.float32(out)

